# revision 41
# baseline (speedup 1.0000x reference)
"""Concordance index kernel for Trainium2 (8 NeuronCores, Bass/Tile).

Math: reference sorts by time (stable), then
  num = sum_i #{ j < i : event_j and risk_j > risk_i }   (i, j in time order)
  den = sum_p e_p * (n-1-p)
  out = num / den

Device computes num (the O(n^2) pairwise part). Host does the O(n log n)
prep: argsort by time, risk ranks, den, and data layout.

Encodings / decomposition:
- risk values -> tie-safe ranks (equal values share a rank), encoded as bf16
  via bit pattern (16384 + rank): strictly monotone, so bf16 `is_gt`
  compares are EXACT, and bf16 enables DVE's 4x perf mode.
- event mask fused into the comparison: sigma_j = event_j ? enc(rank_j) : 0.0
  (0.0 sorts below every encoded rank, so non-events never count).
- row i = 1024*k + 128*c + p  ->  core c, slot k, partition p.  Every core
  runs an IDENTICAL instruction schedule (SPMD + perfect balance).
  The prefix j < i of slot k splits into:
    main(k):  j in [0, 1024k)  unmasked tensor_scalar(is_gt)+accum at 4x
              (DVE) or activation(Sign)+accum (ScalarE, count=(S+N)/2)
    boundary j in [1024k, 1024k + 128c + p), handled one of two ways:
      k in SHIP set: bndF = unmasked 4x count over a shipped per-core
        periodically-zeroed sigma copy (zeros at jj >= 128c, fixed 896-col
        AP) + bndT = 128-wide triangle STT with mask 1[jj < p];
      else: one scalar_tensor_tensor((sig is_gt rho) * stair) at 1x with
        the staircase mask 1[jj < 128c + p].
  Trade-off: shipping bndF data costs +0.25MB DMA per slot but turns a 1x
  1024-col op into a 4x 896-col op + tiny triangle. The SHIP set balances
  the DMA-vs-DVE critical path.
- per-instruction [128,1] fp32 partials are integers; host sums in float64.

Packed per-core input layout (bf16 columns):
  [0:8]          rho_bf (slot k threshold at col k, per partition)
  [8:136]        triangle mask 1[jj < p]
  [136:1160]     staircase mask 1[jj < 128c + p]
  [1160:9352]    sigma[0:8192] (contiguous -> simple main APs)
  [9352 + 1024f] bndF_k(896) | bndT_k(128) for the f-th slot of SHIP

Hardware-constraint notes (hard-won):
- Most instruction formats hold ONE sem wait (DMA copies, control/drain) or
  very few (compute). So: all inputs in ONE tensor moved by <=7 dma_starts
  (+1 output = 8 queues max, fresh queue for the output), compute issued in
  DMA-arrival order (each op adds <=1 new wait), per-engine 1-column
  "funnel" copies collapse DMA-queue sems into program order, SP nops with
  explicit dep edges pre-consume queue/engine sems so the kernel-tail drain
  stays within its wait budget, and the output DMA reads a tile written by
  exactly one instruction.
- GPSIMD cannot execute TensorScalarPtr (ISA check) - no compute there.
"""

import os
import sys

import numpy as np

for _p in ("/opt/trn_rl_repo", "/root/.axon_site/_ro/trn_rl_repo"):
    if os.path.isdir(_p) and _p not in sys.path:
        sys.path.insert(0, _p)

import ml_dtypes  # noqa: E402

N = 8192
NCORES = 8
NSLOTS = 8  # row groups per core; group g = 8*k + c; 128 rows per group
CHUNK = 1024  # j-columns per slot
BF16 = ml_dtypes.bfloat16
ENC_BASE = 16384  # bf16 bit pattern base (value 2.0); +8191 stays finite

SIG0 = 8 + 128 + CHUNK  # header: rho | tri_mask | stair_mask
BND0 = SIG0 + N  # 9352: start of shipped boundary data

# default engine config; tuned via TimelineSim + HW checks
DEFAULT_CFG = {
    # main-slot chunks on ScalarE: k -> number of leading 1024-chunks
    "scalare_chunks": {7: 7, 6: 2},
    "ship": (2, 3, 4, 5, 6, 7),  # slots with shipped bndF/bndT data
    "scalare_bndf": frozenset(),  # bndF slots on ScalarE
}


def _tot_cols(cfg):
    return BND0 + 1024 * len(cfg["ship"])


def _grp_ends(cfg):
    """<=7 input DMA groups: sigma split for pipelining, then bnd data."""
    tot = _tot_cols(cfg)
    ends = [SIG0 + 1024, SIG0 + 3072, SIG0 + 5120, BND0]
    nb = len(cfg["ship"])
    if nb == 0:
        return tuple(ends)
    if nb >= 4:
        ends += [BND0 + 1024 * (nb // 2), tot]
    else:
        ends += [tot]
    return tuple(ends)


def _build_work(cfg):
    """Instruction list: (kind, k, j0, j1, eng) in pipeline issue order.

    kind: 'main' | 'bndF' | 'bndT' | 'bndS' (staircase STT).
    eng: 'v' DVE, 's' ScalarE.
    """
    ship = set(cfg["ship"])
    entries = []
    for k in range(1, NSLOTS):
        ns = min(cfg["scalare_chunks"].get(k, 0), k)
        for u in range(ns):  # per-chunk ScalarE pieces (pipeline with DMA)
            entries.append(((u, 2), ("main", k, u * CHUNK, (u + 1) * CHUNK, "s")))
        if ns < k:
            entries.append(((k - 1, 3), ("main", k, ns * CHUNK, k * CHUNK, "v")))
    for k in range(NSLOTS):
        if k in ship:
            eF = "s" if k in cfg["scalare_bndf"] else "v"
            entries.append(((7, 4, k), ("bndF", k, 0, 896, eF)))
            entries.append(((7, 5, k), ("bndT", k, 0, 128, "v")))
        else:
            entries.append(((k, 1), ("bndS", k, 0, CHUNK, "v")))
    entries.sort(key=lambda e: e[0])
    return [e for _, e in entries]


def _build_program(work, cfg, use_scalare, skip_compute=False, funnels=True):
    import bass_rust
    import concourse.bass as bass
    import concourse.mybir as mybir
    from concourse.tile import TileContext

    dt = mybir.dt
    Alu = mybir.AluOpType
    Act = mybir.ActivationFunctionType

    ship = sorted(cfg["ship"])
    bnd_base = {k: BND0 + 1024 * f for f, k in enumerate(ship)}
    grp_ends = _grp_ends(cfg)
    tot = _tot_cols(cfg)
    nacc = len(work)
    ngrp = len(grp_ends)
    nc = bass.Bass()
    packed_d = nc.declare_dram_parameter("packed", [128, tot], dt.bfloat16, False)
    acc_d = nc.declare_dram_parameter("acc", [128, nacc], dt.float32, True)

    with TileContext(nc) as tc:
        with tc.tile_pool(name="p", bufs=1) as pool:
            big = pool.tile([128, tot], dt.bfloat16)
            rho = pool.tile([128, NSLOTS], dt.float32)
            rhon = pool.tile([128, NSLOTS], dt.float32)
            acc = pool.tile([128, nacc], dt.float32)
            acc2 = pool.tile([128, nacc], dt.float32)
            scr_v = pool.tile([128, (NSLOTS - 1) * CHUNK], dt.bfloat16)
            scr_s = pool.tile([128, (NSLOTS - 1) * CHUNK], dt.bfloat16)
            warm_v = pool.tile([128, ngrp], dt.bfloat16)
            warm_s = pool.tile([128, ngrp], dt.bfloat16)

            tri_mask = big[:, 8 : 8 + 128]
            stair = big[:, 136 : 136 + CHUNK]

            g0 = 0
            dmas = []
            for ge in grp_ends:
                dmas.append(
                    nc.sync.dma_start(out=big[:, g0:ge], in_=packed_d[:, g0:ge])
                )
                g0 = ge

            # per-engine funnels: collapse each DMA group's queue sem into
            # the engine's program order via a 1-column copy
            funneled = {e: [not funnels] * ngrp for e in ("v", "s")}
            warms = {"v": warm_v, "s": warm_s}

            def _funnel(col_abs, eng):
                flags = funneled[eng]
                for g in range(ngrp):
                    gstart = 0 if g == 0 else grp_ends[g - 1]
                    if flags[g] or gstart > col_abs:
                        continue
                    flags[g] = True
                    c = grp_ends[g] - 1
                    if eng == "s":
                        nc.scalar.copy(warms[eng][:, g : g + 1], big[:, c : c + 1])
                    else:
                        nc.vector.tensor_copy(
                            warms[eng][:, g : g + 1], big[:, c : c + 1]
                        )

            # rho (fp32, for is_gt scalar / Sign bias) via converting copy
            _funnel(SIG0, "v")
            nc.vector.tensor_copy(rho[:], big[:, 0:NSLOTS])
            if use_scalare:
                _funnel(SIG0, "s")
                nc.scalar.activation(
                    out=rhon[:], in_=rho[:], func=Act.Copy, scale=-1.0
                )

            last_inst_by_eng = {}
            if skip_compute:
                nc.vector.memset(acc[:], 0.0)
            for idx, (kind, k, j0, j1, eng) in enumerate(work):
                if skip_compute:
                    break
                a = acc[:, idx : idx + 1]
                if kind == "bndT":
                    b = bnd_base[k]
                    _funnel(b + 1024 - 1, "v")
                    inst = nc.vector.scalar_tensor_tensor(
                        out=scr_v[:, :128],
                        in0=big[:, b + 896 : b + 1024],
                        scalar=rho[:, k : k + 1],
                        in1=tri_mask,
                        op0=Alu.is_gt,
                        op1=Alu.mult,
                        accum_out=a,
                    )
                elif kind == "bndS":
                    _funnel(SIG0 + (k + 1) * CHUNK - 1, "v")
                    inst = nc.vector.scalar_tensor_tensor(
                        out=scr_v[:, :CHUNK],
                        in0=big[:, SIG0 + k * CHUNK : SIG0 + (k + 1) * CHUNK],
                        scalar=rho[:, k : k + 1],
                        in1=stair,
                        op0=Alu.is_gt,
                        op1=Alu.mult,
                        accum_out=a,
                    )
                else:
                    if kind == "bndF":
                        b = bnd_base[k]
                        src = big[:, b : b + 896]
                        last_col = b + 896 - 1
                    else:
                        src = big[:, SIG0 + j0 : SIG0 + j1]
                        last_col = SIG0 + j1 - 1
                    L = j1 - j0
                    _funnel(last_col, eng)
                    if eng == "v":
                        inst = nc.vector.tensor_scalar(
                            scr_v[:, :L],
                            src,
                            rho[:, k : k + 1],
                            0.0,
                            Alu.is_gt,
                            Alu.add,
                            accum_out=a,
                        )
                    else:  # ScalarE Sign trick; count = (S + L)/2 host-side
                        inst = nc.scalar.activation(
                            out=scr_s[:, :L],
                            in_=src,
                            func=Act.Sign,
                            bias=rhon[:, k : k + 1],
                            scale=1.0,
                            accum_out=a,
                        )
                last_inst_by_eng[eng] = inst

            # single-writer funnel so the output DMA needs exactly one wait;
            # pre-consume the ScalarE completion sem first (1 wait per copy)
            for e in last_inst_by_eng:
                if e != "v":
                    nc.vector.tensor_copy(acc2[:, :1], acc[:, :1])
            nc.vector.tensor_copy(acc2[:], acc[:])
            dmas.append(nc.sync.dma_start(out=acc_d[:], in_=acc2[:]))

            # kernel-tail drain holds very few waits: pre-consume queue and
            # engine sems on the SP proc via nops with explicit dep edges
            for e, inst in last_inst_by_eng.items():
                if e != "v":
                    nop = nc.sync.nop(nofuse=True)
                    bass_rust.add_dep_helper(
                        nop.ins, inst.ins, reason="spread drain engine waits"
                    )
            for d in dmas:
                nop = nc.sync.nop(nofuse=True)
                bass_rust.add_dep_helper(
                    nop.ins, d.ins, reason="spread drain queue waits"
                )
    return nc


def _prepare(risk, time, event, cfg):
    order = np.argsort(time, kind="stable")
    r = np.asarray(risk)[order]
    e = np.asarray(event)[order]

    # tie-safe ranks: equal risks share a rank so strict is_gt stays exact
    rk = np.searchsorted(np.sort(r), r, side="left").astype(np.int32)
    has_ties = bool(np.unique(r).size != r.size)

    enc_bits = (ENC_BASE + rk).astype(np.uint16)
    sig_bits = np.where(e > 0, enc_bits, np.uint16(0))  # [N] uint16

    # rho[p, k] for core c: row i = 1024k + 128c + p
    rho_all = enc_bits.reshape(NSLOTS, NCORES, 128)  # [k, c, p]

    ship = sorted(cfg["ship"])
    tot = _tot_cols(cfg)
    p_idx = np.arange(128)[:, None]
    jj128 = np.arange(128)[None, :]
    jj1024 = np.arange(CHUNK)[None, :]
    one = np.uint16(0x3F80)  # bf16 1.0 bit pattern

    in_maps = []
    for c in range(NCORES):
        pk = np.zeros((128, tot), dtype=np.uint16)
        pk[:, 0:NSLOTS] = rho_all[:, c, :].T
        pk[:, 8:136] = (jj128 < p_idx).astype(np.uint16) * one
        pk[:, 136:SIG0] = (jj1024 < 128 * c + p_idx).astype(np.uint16) * one
        pk[:, SIG0:BND0] = sig_bits[None, :]
        w = 128 * c
        for f, k in enumerate(ship):
            b = BND0 + 1024 * f
            pk[:, b : b + w] = sig_bits[None, k * CHUNK : k * CHUNK + w]
            pk[:, b + 896 : b + 1024] = sig_bits[
                None, k * CHUNK + w : k * CHUNK + w + 128
            ]
        in_maps.append({"packed": pk.view(BF16)})

    den = float(np.sum(e.astype(np.float64) * (N - 1 - np.arange(N))))
    return in_maps, den, has_ties


def _reduce(results, work):
    num = 0.0
    for rmap in results:
        a = rmap["acc"].astype(np.float64)  # [128, nacc]
        for idx, (kind, k, j0, j1, eng) in enumerate(work):
            col = a[:, idx]
            if eng == "s":
                num += float(np.sum(col + (j1 - j0)) / 2.0)
            else:
                num += float(np.sum(col))
    return num


def kernel(risk, time, event, _trace=False, _cfg=None):
    from concourse.bass_utils import run_bass_kernel_spmd

    cfg = dict(DEFAULT_CFG)
    if _cfg:
        cfg.update(_cfg)
    in_maps, den, has_ties = _prepare(risk, time, event, cfg)
    if has_ties:
        cfg["scalare_chunks"] = {}  # Sign trick miscounts exact ties by 0.5
        cfg["scalare_bndf"] = frozenset()
    work = _build_work(cfg)
    nc = _build_program(
        work, cfg, use_scalare=any(w[4] == "s" for w in work), funnels=True
    )
    res = run_bass_kernel_spmd(nc, in_maps, list(range(NCORES)), trace=_trace)
    num = _reduce(res.results, work)

    if den == 0.0:
        out = np.float32(np.nan)
    else:
        out = np.float32(num / den)
    if _trace:
        return np.asarray(out, dtype=np.float32), res
    return np.asarray(out, dtype=np.float32)


# revision 43
# speedup vs baseline: 1.0104x; 1.0104x over previous
"""Concordance index kernel for Trainium2 (8 NeuronCores, Bass/Tile).

Math: reference sorts by time (stable), then
  num = sum_i #{ j < i : event_j and risk_j > risk_i }   (i, j in time order)
  den = sum_p e_p * (n-1-p)
  out = num / den

Device computes num (the O(n^2) pairwise part). Host does the O(n log n)
prep: argsort by time, risk ranks, den, and data layout.

Encodings / decomposition:
- risk values -> tie-safe ranks (equal values share a rank), encoded as bf16
  via bit pattern (16384 + rank): strictly monotone, so bf16 `is_gt`
  compares are EXACT, and bf16 enables DVE's 4x perf mode.
- event mask fused into the comparison: sigma_j = event_j ? enc(rank_j) : 0.0
  (0.0 sorts below every encoded rank, so non-events never count).
- row i = 1024*k + 128*c + p  ->  core c, slot k, partition p.  Every core
  runs an IDENTICAL instruction schedule (SPMD + perfect balance).
  The prefix j < i of slot k splits into:
    main(k):  j in [0, 1024k)  unmasked tensor_scalar(is_gt)+accum at 4x
              (DVE) or activation(Sign)+accum (ScalarE, count=(S+N)/2)
    boundary j in [1024k, 1024k + 128c + p), handled one of two ways:
      k in SHIP set: bndF = unmasked 4x count over a shipped per-core
        periodically-zeroed sigma copy (zeros at jj >= 128c, fixed 896-col
        AP) + bndT = 128-wide triangle STT with mask 1[jj < p];
      else: one scalar_tensor_tensor((sig is_gt rho) * stair) at 1x with
        the staircase mask 1[jj < 128c + p].
  Trade-off: shipping bndF data costs +0.25MB DMA per slot but turns a 1x
  1024-col op into a 4x 896-col op + tiny triangle. The SHIP set balances
  the DMA-vs-DVE critical path.
- per-instruction [128,1] fp32 partials are integers; host sums in float64.

Packed per-core input layout (bf16 columns):
  [0:8]          rho_bf (slot k threshold at col k, per partition)
  [8:136]        triangle mask 1[jj < p]
  [136:1160]     staircase mask 1[jj < 128c + p]
  [1160:9352]    sigma[0:8192] (contiguous -> simple main APs)
  [9352 + 1024f] bndF_k(896) | bndT_k(128) for the f-th slot of SHIP

Hardware-constraint notes (hard-won):
- Most instruction formats hold ONE sem wait (DMA copies, control/drain) or
  very few (compute). So: all inputs in ONE tensor moved by <=7 dma_starts
  (+1 output = 8 queues max, fresh queue for the output), compute issued in
  DMA-arrival order (each op adds <=1 new wait), per-engine 1-column
  "funnel" copies collapse DMA-queue sems into program order, SP nops with
  explicit dep edges pre-consume queue/engine sems so the kernel-tail drain
  stays within its wait budget, and the output DMA reads a tile written by
  exactly one instruction.
- GPSIMD cannot execute TensorScalarPtr (ISA check) - no compute there.
"""

import os
import sys

import numpy as np

for _p in ("/opt/trn_rl_repo", "/root/.axon_site/_ro/trn_rl_repo"):
    if os.path.isdir(_p) and _p not in sys.path:
        sys.path.insert(0, _p)

import ml_dtypes  # noqa: E402

N = 8192
NCORES = 8
NSLOTS = 8  # row groups per core; group g = 8*k + c; 128 rows per group
CHUNK = 1024  # j-columns per slot
BF16 = ml_dtypes.bfloat16
ENC_BASE = 16384  # bf16 bit pattern base (value 2.0); +8191 stays finite

SIG0 = 8 + 128 + CHUNK  # header: rho | tri_mask | stair_mask
BND0 = SIG0 + N  # 9352: start of shipped boundary data

# default engine config; tuned via TimelineSim + HW checks
DEFAULT_CFG = {
    # main-slot chunks on ScalarE: k -> number of leading 1024-chunks
    "scalare_chunks": {7: 7, 6: 2},
    "ship": (2, 3, 4, 5, 6, 7),  # slots with shipped bndF/bndT data
    "scalare_bndf": frozenset(),  # bndF slots on ScalarE
    # input DMA split (ramp-friendly: small first group); None -> heuristic
    "grp_ends": (
        SIG0 + 512,
        SIG0 + 1536,
        SIG0 + 3584,
        SIG0 + 6144,
        BND0,
        BND0 + 3072,
    ),
}


def _tot_cols(cfg):
    return BND0 + 1024 * len(cfg["ship"])


def _grp_ends(cfg):
    """<=7 input DMA groups: sigma split for pipelining, then bnd data."""
    tot = _tot_cols(cfg)
    if cfg.get("grp_ends"):
        return tuple(min(e, tot) for e in cfg["grp_ends"] if e <= tot) + (
            (tot,) if cfg["grp_ends"][-1] < tot else ()
        )
    ends = [SIG0 + 1024, SIG0 + 3072, SIG0 + 5120, BND0]
    nb = len(cfg["ship"])
    if nb == 0:
        return tuple(ends)
    if nb >= 4:
        ends += [BND0 + 1024 * (nb // 2), tot]
    else:
        ends += [tot]
    return tuple(ends)


def _build_work(cfg):
    """Instruction list: (kind, k, j0, j1, eng) in pipeline issue order.

    kind: 'main' | 'bndF' | 'bndT' | 'bndS' (staircase STT).
    eng: 'v' DVE, 's' ScalarE.
    """
    ship = set(cfg["ship"])
    entries = []
    for k in range(1, NSLOTS):
        ns = min(cfg["scalare_chunks"].get(k, 0), k)
        for u in range(ns):  # per-chunk ScalarE pieces (pipeline with DMA)
            entries.append(((u, 2), ("main", k, u * CHUNK, (u + 1) * CHUNK, "s")))
        if ns < k:
            entries.append(((k - 1, 3), ("main", k, ns * CHUNK, k * CHUNK, "v")))
    for k in range(NSLOTS):
        if k in ship:
            eF = "s" if k in cfg["scalare_bndf"] else "v"
            entries.append(((7, 4, k), ("bndF", k, 0, 896, eF)))
            entries.append(((7, 5, k), ("bndT", k, 0, 128, "v")))
        else:
            entries.append(((k, 1), ("bndS", k, 0, CHUNK, "v")))
    entries.sort(key=lambda e: e[0])
    return [e for _, e in entries]


def _build_program(work, cfg, use_scalare, skip_compute=False, funnels=True):
    import bass_rust
    import concourse.bass as bass
    import concourse.mybir as mybir
    from concourse.tile import TileContext

    dt = mybir.dt
    Alu = mybir.AluOpType
    Act = mybir.ActivationFunctionType

    ship = sorted(cfg["ship"])
    bnd_base = {k: BND0 + 1024 * f for f, k in enumerate(ship)}
    grp_ends = _grp_ends(cfg)
    tot = _tot_cols(cfg)
    nacc = len(work)
    ngrp = len(grp_ends)
    nc = bass.Bass()
    packed_d = nc.declare_dram_parameter("packed", [128, tot], dt.bfloat16, False)
    acc_d = nc.declare_dram_parameter("acc", [128, nacc], dt.float32, True)

    with TileContext(nc) as tc:
        with tc.tile_pool(name="p", bufs=1) as pool:
            big = pool.tile([128, tot], dt.bfloat16)
            rho = pool.tile([128, NSLOTS], dt.float32)
            rhon = pool.tile([128, NSLOTS], dt.float32)
            acc = pool.tile([128, nacc], dt.float32)
            acc2 = pool.tile([128, nacc], dt.float32)
            scr_v = pool.tile([128, (NSLOTS - 1) * CHUNK], dt.bfloat16)
            scr_s = pool.tile([128, (NSLOTS - 1) * CHUNK], dt.bfloat16)
            warm_v = pool.tile([128, ngrp], dt.bfloat16)
            warm_s = pool.tile([128, ngrp], dt.bfloat16)

            tri_mask = big[:, 8 : 8 + 128]
            stair = big[:, 136 : 136 + CHUNK]

            g0 = 0
            dmas = []
            for ge in grp_ends:
                dmas.append(
                    nc.sync.dma_start(out=big[:, g0:ge], in_=packed_d[:, g0:ge])
                )
                g0 = ge

            # per-engine funnels: collapse each DMA group's queue sem into
            # the engine's program order via a 1-column copy
            funneled = {e: [not funnels] * ngrp for e in ("v", "s")}
            warms = {"v": warm_v, "s": warm_s}

            def _funnel(col_abs, eng):
                flags = funneled[eng]
                for g in range(ngrp):
                    gstart = 0 if g == 0 else grp_ends[g - 1]
                    if flags[g] or gstart > col_abs:
                        continue
                    flags[g] = True
                    c = grp_ends[g] - 1
                    if eng == "s":
                        nc.scalar.copy(warms[eng][:, g : g + 1], big[:, c : c + 1])
                    else:
                        nc.vector.tensor_copy(
                            warms[eng][:, g : g + 1], big[:, c : c + 1]
                        )

            # rho (fp32, for is_gt scalar / Sign bias) via converting copy
            _funnel(SIG0, "v")
            nc.vector.tensor_copy(rho[:], big[:, 0:NSLOTS])
            if use_scalare:
                _funnel(SIG0, "s")
                nc.scalar.activation(
                    out=rhon[:], in_=rho[:], func=Act.Copy, scale=-1.0
                )

            last_inst_by_eng = {}
            if skip_compute:
                nc.vector.memset(acc[:], 0.0)
            for idx, (kind, k, j0, j1, eng) in enumerate(work):
                if skip_compute:
                    break
                a = acc[:, idx : idx + 1]
                if kind == "bndT":
                    b = bnd_base[k]
                    _funnel(b + 1024 - 1, "v")
                    inst = nc.vector.scalar_tensor_tensor(
                        out=scr_v[:, :128],
                        in0=big[:, b + 896 : b + 1024],
                        scalar=rho[:, k : k + 1],
                        in1=tri_mask,
                        op0=Alu.is_gt,
                        op1=Alu.mult,
                        accum_out=a,
                    )
                elif kind == "bndS":
                    _funnel(SIG0 + (k + 1) * CHUNK - 1, "v")
                    inst = nc.vector.scalar_tensor_tensor(
                        out=scr_v[:, :CHUNK],
                        in0=big[:, SIG0 + k * CHUNK : SIG0 + (k + 1) * CHUNK],
                        scalar=rho[:, k : k + 1],
                        in1=stair,
                        op0=Alu.is_gt,
                        op1=Alu.mult,
                        accum_out=a,
                    )
                else:
                    if kind == "bndF":
                        b = bnd_base[k]
                        src = big[:, b : b + 896]
                        last_col = b + 896 - 1
                    else:
                        src = big[:, SIG0 + j0 : SIG0 + j1]
                        last_col = SIG0 + j1 - 1
                    L = j1 - j0
                    _funnel(last_col, eng)
                    if eng == "v":
                        inst = nc.vector.tensor_scalar(
                            scr_v[:, :L],
                            src,
                            rho[:, k : k + 1],
                            0.0,
                            Alu.is_gt,
                            Alu.add,
                            accum_out=a,
                        )
                    else:  # ScalarE Sign trick; count = (S + L)/2 host-side
                        inst = nc.scalar.activation(
                            out=scr_s[:, :L],
                            in_=src,
                            func=Act.Sign,
                            bias=rhon[:, k : k + 1],
                            scale=1.0,
                            accum_out=a,
                        )
                last_inst_by_eng[eng] = inst

            # single-writer funnel so the output DMA needs exactly one wait;
            # pre-consume the ScalarE completion sem first (1 wait per copy)
            for e in last_inst_by_eng:
                if e != "v":
                    nc.vector.tensor_copy(acc2[:, :1], acc[:, :1])
            nc.vector.tensor_copy(acc2[:], acc[:])
            dmas.append(nc.sync.dma_start(out=acc_d[:], in_=acc2[:]))

            # kernel-tail drain holds very few waits: pre-consume queue and
            # engine sems on the SP proc via nops with explicit dep edges
            for e, inst in last_inst_by_eng.items():
                if e != "v":
                    nop = nc.sync.nop(nofuse=True)
                    bass_rust.add_dep_helper(
                        nop.ins, inst.ins, reason="spread drain engine waits"
                    )
            for d in dmas:
                nop = nc.sync.nop(nofuse=True)
                bass_rust.add_dep_helper(
                    nop.ins, d.ins, reason="spread drain queue waits"
                )
    return nc


def _prepare(risk, time, event, cfg):
    order = np.argsort(time, kind="stable")
    r = np.asarray(risk)[order]
    e = np.asarray(event)[order]

    # tie-safe ranks: equal risks share a rank so strict is_gt stays exact
    rk = np.searchsorted(np.sort(r), r, side="left").astype(np.int32)
    has_ties = bool(np.unique(r).size != r.size)

    enc_bits = (ENC_BASE + rk).astype(np.uint16)
    sig_bits = np.where(e > 0, enc_bits, np.uint16(0))  # [N] uint16

    # rho[p, k] for core c: row i = 1024k + 128c + p
    rho_all = enc_bits.reshape(NSLOTS, NCORES, 128)  # [k, c, p]

    ship = sorted(cfg["ship"])
    tot = _tot_cols(cfg)
    p_idx = np.arange(128)[:, None]
    jj128 = np.arange(128)[None, :]
    jj1024 = np.arange(CHUNK)[None, :]
    one = np.uint16(0x3F80)  # bf16 1.0 bit pattern

    in_maps = []
    for c in range(NCORES):
        pk = np.zeros((128, tot), dtype=np.uint16)
        pk[:, 0:NSLOTS] = rho_all[:, c, :].T
        pk[:, 8:136] = (jj128 < p_idx).astype(np.uint16) * one
        pk[:, 136:SIG0] = (jj1024 < 128 * c + p_idx).astype(np.uint16) * one
        pk[:, SIG0:BND0] = sig_bits[None, :]
        w = 128 * c
        for f, k in enumerate(ship):
            b = BND0 + 1024 * f
            pk[:, b : b + w] = sig_bits[None, k * CHUNK : k * CHUNK + w]
            pk[:, b + 896 : b + 1024] = sig_bits[
                None, k * CHUNK + w : k * CHUNK + w + 128
            ]
        in_maps.append({"packed": pk.view(BF16)})

    den = float(np.sum(e.astype(np.float64) * (N - 1 - np.arange(N))))
    return in_maps, den, has_ties


def _reduce(results, work):
    num = 0.0
    for rmap in results:
        a = rmap["acc"].astype(np.float64)  # [128, nacc]
        for idx, (kind, k, j0, j1, eng) in enumerate(work):
            col = a[:, idx]
            if eng == "s":
                num += float(np.sum(col + (j1 - j0)) / 2.0)
            else:
                num += float(np.sum(col))
    return num


def kernel(risk, time, event, _trace=False, _cfg=None):
    from concourse.bass_utils import run_bass_kernel_spmd

    cfg = dict(DEFAULT_CFG)
    if _cfg:
        cfg.update(_cfg)
    in_maps, den, has_ties = _prepare(risk, time, event, cfg)
    if has_ties:
        cfg["scalare_chunks"] = {}  # Sign trick miscounts exact ties by 0.5
        cfg["scalare_bndf"] = frozenset()
    work = _build_work(cfg)
    nc = _build_program(
        work, cfg, use_scalare=any(w[4] == "s" for w in work), funnels=True
    )
    res = run_bass_kernel_spmd(nc, in_maps, list(range(NCORES)), trace=_trace)
    num = _reduce(res.results, work)

    if den == 0.0:
        out = np.float32(np.nan)
    else:
        out = np.float32(num / den)
    if _trace:
        return np.asarray(out, dtype=np.float32), res
    return np.asarray(out, dtype=np.float32)


# revision 52
# speedup vs baseline: 1.0926x; 1.0813x over previous
"""Concordance index kernel for Trainium2 (8 NeuronCores, Bass/Tile).

Math: reference sorts by time (stable), then
  num = sum_i #{ j < i : event_j and risk_j > risk_i }   (i, j in time order)
  den = sum_p e_p * (n-1-p)
  out = num / den

Device computes num (the O(n^2) pairwise part). Host does the O(n log n)
prep: argsort by time, risk ranks, den, and data layout.

Encodings / decomposition:
- risk values -> tie-safe ranks (equal values share a rank), encoded as bf16
  via bit pattern (16384 + rank): strictly monotone, so bf16 `is_gt`
  compares are EXACT, and bf16 enables DVE's 4x perf mode.
- event mask fused into the comparison: sigma_j = event_j ? enc(rank_j) : 0.0
  (0.0 sorts below every encoded rank, so non-events never count).
- row i = 1024*k + 128*c + p  ->  core c, slot k, partition p.  Every core
  runs an IDENTICAL instruction schedule (SPMD + perfect balance).
  The prefix j < i of slot k splits into:
    main(k):  j in [0, 1024k)  unmasked tensor_scalar(is_gt)+accum at 4x
              (DVE) or activation(Sign)+accum (ScalarE, count=(S+N)/2)
    boundary j in [1024k, 1024k + 128c + p), handled one of two ways:
      k in SHIP set: bndF = unmasked 4x count over a shipped per-core
        periodically-zeroed sigma copy (zeros at jj >= 128c, fixed 896-col
        AP) + bndT = 128-wide triangle STT with mask 1[jj < p];
      else: one scalar_tensor_tensor((sig is_gt rho) * stair) at 1x with
        the staircase mask 1[jj < 128c + p].
  Trade-off: shipping bndF data costs +0.25MB DMA per slot but turns a 1x
  1024-col op into a 4x 896-col op + tiny triangle. The SHIP set balances
  the DMA-vs-DVE critical path.
- per-instruction [128,1] fp32 partials are integers; host sums in float64.

Packed per-core input layout (bf16 columns):
  [0:8]          rho_bf (slot k threshold at col k, per partition)
  [8:136]        triangle mask 1[jj < p]
  [136:1160]     staircase mask 1[jj < 128c + p]
  [1160:9352]    sigma[0:8192] (contiguous -> simple main APs)
  [9352 + 1024f] bndF_k(896) | bndT_k(128) for the f-th slot of SHIP

Hardware-constraint notes (hard-won):
- Most instruction formats hold ONE sem wait (DMA copies, control/drain) or
  very few (compute). So: all inputs in ONE tensor moved by <=7 dma_starts
  (+1 output = 8 queues max, fresh queue for the output), compute issued in
  DMA-arrival order (each op adds <=1 new wait), per-engine 1-column
  "funnel" copies collapse DMA-queue sems into program order, SP nops with
  explicit dep edges pre-consume queue/engine sems so the kernel-tail drain
  stays within its wait budget, and the output DMA reads a tile written by
  exactly one instruction.
- GPSIMD cannot execute TensorScalarPtr (ISA check) - no compute there.
"""

import os
import sys

import numpy as np

for _p in ("/opt/trn_rl_repo", "/root/.axon_site/_ro/trn_rl_repo"):
    if os.path.isdir(_p) and _p not in sys.path:
        sys.path.insert(0, _p)

import ml_dtypes  # noqa: E402

N = 8192
NCORES = 8
NSLOTS = 8  # row groups per core; group g = 8*k + c; 128 rows per group
CHUNK = 1024  # j-columns per slot
BF16 = ml_dtypes.bfloat16
ENC_BASE = 16384  # bf16 bit pattern base (value 2.0); +8191 stays finite

SIG0 = 8 + 128 + CHUNK  # header: rho | tri_mask | stair_mask
BND0 = SIG0 + N  # 9352: start of shipped boundary data

# default engine config; tuned via TimelineSim + HW checks
DEFAULT_CFG = {
    # main-slot chunks on ScalarE: k -> number of leading 1024-chunks
    "scalare_chunks": {7: 7, 6: 2},
    "s_merge_from": 2,  # ScalarE chunks >= this index merge into one op
    "ship": (2, 3, 4, 5, 6, 7),  # slots with shipped bndF/bndT data
    "scalare_bndf": frozenset(),  # bndF slots on ScalarE
    "raw": True,  # raw Block program (no Tile scheduling/tail overhead)
    # input DMA split (ramp-friendly: small first group); None -> heuristic
    "grp_ends": (
        SIG0 + 512,
        SIG0 + 1536,
        SIG0 + 2560,
        SIG0 + 3584,
        SIG0 + 5120,
        SIG0 + 6656,
        BND0,
        BND0 + 2048,
        BND0 + 4096,
    ),
}


def _tot_cols(cfg):
    return BND0 + 1024 * len(cfg["ship"])


def _grp_ends(cfg):
    """<=7 input DMA groups: sigma split for pipelining, then bnd data."""
    tot = _tot_cols(cfg)
    if cfg.get("grp_ends"):
        return tuple(min(e, tot) for e in cfg["grp_ends"] if e <= tot) + (
            (tot,) if cfg["grp_ends"][-1] < tot else ()
        )
    ends = [SIG0 + 1024, SIG0 + 3072, SIG0 + 5120, BND0]
    nb = len(cfg["ship"])
    if nb == 0:
        return tuple(ends)
    if nb >= 4:
        ends += [BND0 + 1024 * (nb // 2), tot]
    else:
        ends += [tot]
    return tuple(ends)


def _build_work(cfg):
    """Instruction list: (kind, k, j0, j1, eng) in pipeline issue order.

    kind: 'main' | 'bndF' | 'bndT' | 'bndS' (staircase STT).
    eng: 'v' DVE, 's' ScalarE.
    """
    ship = set(cfg["ship"])
    merge_from = cfg.get("s_merge_from", NSLOTS)  # chunks >= this merge
    entries = []
    for k in range(1, NSLOTS):
        ns = min(cfg["scalare_chunks"].get(k, 0), k)
        for u in range(min(ns, merge_from)):  # per-chunk pieces (pipeline)
            entries.append(((u, 2), ("main", k, u * CHUNK, (u + 1) * CHUNK, "s")))
        if ns > merge_from:  # tail chunks merged into one ACT op
            entries.append(
                ((merge_from, 2), ("main", k, merge_from * CHUNK, ns * CHUNK, "s"))
            )
        if ns < k:
            entries.append(((k - 1, 3), ("main", k, ns * CHUNK, k * CHUNK, "v")))
    for k in range(NSLOTS):
        if k in ship:
            eF = "s" if k in cfg["scalare_bndf"] else "v"
            entries.append(((7, 4, k), ("bndF", k, 0, 896, eF)))
            entries.append(((7, 5, k), ("bndT", k, 0, 128, "v")))
        else:
            entries.append(((k, 1), ("bndS", k, 0, CHUNK, "v")))
    entries.sort(key=lambda e: e[0])
    return [e for _, e in entries]


def _build_program(work, cfg, use_scalare, skip_compute=False, funnels=True):
    import bass_rust
    import concourse.bass as bass
    import concourse.mybir as mybir
    from concourse.tile import TileContext

    dt = mybir.dt
    Alu = mybir.AluOpType
    Act = mybir.ActivationFunctionType

    ship = sorted(cfg["ship"])
    bnd_base = {k: BND0 + 1024 * f for f, k in enumerate(ship)}
    grp_ends = _grp_ends(cfg)
    tot = _tot_cols(cfg)
    nacc = len(work)
    ngrp = len(grp_ends)
    nc = bass.Bass()
    packed_d = nc.declare_dram_parameter("packed", [128, tot], dt.bfloat16, False)
    acc_d = nc.declare_dram_parameter("acc", [128, nacc], dt.float32, True)

    with TileContext(nc) as tc:
        with tc.tile_pool(name="p", bufs=1) as pool:
            big = pool.tile([128, tot], dt.bfloat16)
            rho = pool.tile([128, NSLOTS], dt.float32)
            rhon = pool.tile([128, NSLOTS], dt.float32)
            acc = pool.tile([128, nacc], dt.float32)
            acc2 = pool.tile([128, nacc], dt.float32)
            scr_v = pool.tile([128, (NSLOTS - 1) * CHUNK], dt.bfloat16)
            scr_s = pool.tile([128, (NSLOTS - 1) * CHUNK], dt.bfloat16)
            warm_v = pool.tile([128, ngrp], dt.bfloat16)
            warm_s = pool.tile([128, ngrp], dt.bfloat16)

            tri_mask = big[:, 8 : 8 + 128]
            stair = big[:, 136 : 136 + CHUNK]

            g0 = 0
            dmas = []
            for ge in grp_ends:
                dmas.append(
                    nc.sync.dma_start(out=big[:, g0:ge], in_=packed_d[:, g0:ge])
                )
                g0 = ge

            # per-engine funnels: collapse each DMA group's queue sem into
            # the engine's program order via a 1-column copy
            funneled = {e: [not funnels] * ngrp for e in ("v", "s")}
            warms = {"v": warm_v, "s": warm_s}

            def _funnel(col_abs, eng):
                flags = funneled[eng]
                for g in range(ngrp):
                    gstart = 0 if g == 0 else grp_ends[g - 1]
                    if flags[g] or gstart > col_abs:
                        continue
                    flags[g] = True
                    c = grp_ends[g] - 1
                    if eng == "s":
                        nc.scalar.copy(warms[eng][:, g : g + 1], big[:, c : c + 1])
                    else:
                        nc.vector.tensor_copy(
                            warms[eng][:, g : g + 1], big[:, c : c + 1]
                        )

            # rho (fp32, for is_gt scalar / Sign bias) via converting copy
            _funnel(SIG0, "v")
            nc.vector.tensor_copy(rho[:], big[:, 0:NSLOTS])
            if use_scalare:
                _funnel(SIG0, "s")
                nc.scalar.activation(
                    out=rhon[:], in_=rho[:], func=Act.Copy, scale=-1.0
                )

            last_inst_by_eng = {}
            if skip_compute:
                nc.vector.memset(acc[:], 0.0)
            for idx, (kind, k, j0, j1, eng) in enumerate(work):
                if skip_compute:
                    break
                a = acc[:, idx : idx + 1]
                if kind == "bndT":
                    b = bnd_base[k]
                    _funnel(b + 1024 - 1, "v")
                    inst = nc.vector.scalar_tensor_tensor(
                        out=scr_v[:, :128],
                        in0=big[:, b + 896 : b + 1024],
                        scalar=rho[:, k : k + 1],
                        in1=tri_mask,
                        op0=Alu.is_gt,
                        op1=Alu.mult,
                        accum_out=a,
                    )
                elif kind == "bndS":
                    _funnel(SIG0 + (k + 1) * CHUNK - 1, "v")
                    inst = nc.vector.scalar_tensor_tensor(
                        out=scr_v[:, :CHUNK],
                        in0=big[:, SIG0 + k * CHUNK : SIG0 + (k + 1) * CHUNK],
                        scalar=rho[:, k : k + 1],
                        in1=stair,
                        op0=Alu.is_gt,
                        op1=Alu.mult,
                        accum_out=a,
                    )
                else:
                    if kind == "bndF":
                        b = bnd_base[k]
                        src = big[:, b : b + 896]
                        last_col = b + 896 - 1
                    else:
                        src = big[:, SIG0 + j0 : SIG0 + j1]
                        last_col = SIG0 + j1 - 1
                    L = j1 - j0
                    _funnel(last_col, eng)
                    if eng == "v":
                        inst = nc.vector.tensor_scalar(
                            scr_v[:, :L],
                            src,
                            rho[:, k : k + 1],
                            0.0,
                            Alu.is_gt,
                            Alu.add,
                            accum_out=a,
                        )
                    else:  # ScalarE Sign trick; count = (S + L)/2 host-side
                        inst = nc.scalar.activation(
                            out=scr_s[:, :L],
                            in_=src,
                            func=Act.Sign,
                            bias=rhon[:, k : k + 1],
                            scale=1.0,
                            accum_out=a,
                        )
                last_inst_by_eng[eng] = inst

            # single-writer funnel so the output DMA needs exactly one wait;
            # pre-consume the ScalarE completion sem first (1 wait per copy)
            for e in last_inst_by_eng:
                if e != "v":
                    nc.vector.tensor_copy(acc2[:, :1], acc[:, :1])
            nc.vector.tensor_copy(acc2[:], acc[:])
            dmas.append(nc.sync.dma_start(out=acc_d[:], in_=acc2[:]))

            # kernel-tail drain holds very few waits: pre-consume queue and
            # engine sems on the SP proc via nops with explicit dep edges
            for e, inst in last_inst_by_eng.items():
                if e != "v":
                    nop = nc.sync.nop(nofuse=True)
                    bass_rust.add_dep_helper(
                        nop.ins, inst.ins, reason="spread drain engine waits"
                    )
            for d in dmas:
                nop = nc.sync.nop(nofuse=True)
                bass_rust.add_dep_helper(
                    nop.ins, d.ins, reason="spread drain queue waits"
                )
    return nc


def _build_program_raw(work, cfg, use_scalare):
    """Raw Block-mode program: explicit per-engine streams + semaphores.

    Skips TileContext's scheduling and its expensive kernel-tail drain +
    barrier. Each `wait_ge` is its own instruction, so the tiny per-format
    sem-wait budgets stop mattering. One semaphore per input DMA group
    (queue completions are out-of-order), plus rho-staging and per-engine
    completion sems gating the output DMA.
    """
    import concourse.bass as bass
    import concourse.mybir as mybir

    dt = mybir.dt
    Alu = mybir.AluOpType
    Act = mybir.ActivationFunctionType

    ship = sorted(cfg["ship"])
    bnd_base = {k: BND0 + 1024 * f for f, k in enumerate(ship)}
    grp_ends = _grp_ends(cfg)
    tot = _tot_cols(cfg)
    nacc = len(work)
    ngrp = len(grp_ends)

    nc = bass.Bass()
    packed_d = nc.declare_dram_parameter("packed", [128, tot], dt.bfloat16, False)
    acc_d = nc.declare_dram_parameter("acc", [128, nacc], dt.float32, True)

    with (
        nc.sbuf_tensor("big", [128, tot], dt.bfloat16) as big,
        nc.sbuf_tensor("rho", [128, NSLOTS], dt.float32) as rho,
        nc.sbuf_tensor("rhon", [128, NSLOTS], dt.float32) as rhon,
        nc.sbuf_tensor("acc_sb", [128, nacc], dt.float32) as acc,
        nc.sbuf_tensor("scr_v", [128, (NSLOTS - 1) * CHUNK], dt.bfloat16) as scr_v,
        nc.sbuf_tensor("scr_s", [128, (NSLOTS - 1) * CHUNK], dt.bfloat16) as scr_s,
    ):
        sems = [nc.semaphore(f"g{g}") for g in range(ngrp)]
        g_sem = [s.__enter__() for s in sems]
        rho_done = nc.semaphore("rho_done").__enter__()
        vdone = nc.semaphore("vdone").__enter__()
        sdone = nc.semaphore("sdone").__enter__()
        odone = nc.semaphore("odone").__enter__()

        def grp_of(col):
            for g, ge in enumerate(grp_ends):
                if col < ge:
                    return g
            return ngrp - 1

        v_work = [w for w in work if w[4] == "v"]
        s_work = [w for w in work if w[4] == "s"]

        def col_range_of(w):
            """(first, last) input columns an op reads (besides rho)."""
            kind, k, j0, j1, eng = w
            if kind == "bndT":
                return (8, bnd_base[k] + 1024 - 1)  # tri mask + bndT cols
            if kind == "bndF":
                return (bnd_base[k], bnd_base[k] + 896 - 1)
            if kind == "bndS":
                return (136, SIG0 + (k + 1) * CHUNK - 1)  # stair + chunk
            return (SIG0 + j0, SIG0 + j1 - 1)

        with nc.Block() as block:

            @block.sync
            def _(sync):
                spans = []
                g0 = 0
                for ge in grp_ends:
                    spans.append((g0, ge))
                    g0 = ge
                order = cfg.get("dma_order") or range(ngrp)
                for g in order:
                    a0, a1 = spans[g]
                    sync.dma_start(
                        out=big[:, a0:a1], in_=packed_d[:, a0:a1]
                    ).then_inc(g_sem[g], 16)
                sync.wait_ge(vdone, 1)
                if use_scalare:
                    sync.wait_ge(sdone, 1)
                sync.dma_start(out=acc_d[:], in_=acc[:]).then_inc(odone, 16)
                sync.wait_ge(odone, 16)

            @block.vector
            def _(vector):
                waited = set()

                def need(c0, c1):
                    for g in range(grp_of(c0), grp_of(c1) + 1):
                        if g not in waited:
                            waited.add(g)
                            vector.wait_ge(g_sem[g], 16)

                need(0, 0)
                vector.tensor_copy(rho[:], big[:, 0:NSLOTS]).then_inc(rho_done, 1)
                last = None
                for w in v_work:
                    kind, k, j0, j1, eng = w
                    idx = work.index(w)
                    a = acc[:, idx : idx + 1]
                    need(*col_range_of(w))
                    if kind == "bndT":
                        b = bnd_base[k]
                        last = vector.scalar_tensor_tensor(
                            out=scr_v[:, :128],
                            in0=big[:, b + 896 : b + 1024],
                            scalar=rho[:, k : k + 1],
                            in1=big[:, 8 : 8 + 128],
                            op0=Alu.is_gt,
                            op1=Alu.mult,
                            accum_out=a,
                        )
                    elif kind == "bndS":
                        last = vector.scalar_tensor_tensor(
                            out=scr_v[:, :CHUNK],
                            in0=big[:, SIG0 + k * CHUNK : SIG0 + (k + 1) * CHUNK],
                            scalar=rho[:, k : k + 1],
                            in1=big[:, 136 : 136 + CHUNK],
                            op0=Alu.is_gt,
                            op1=Alu.mult,
                            accum_out=a,
                        )
                    else:
                        if kind == "bndF":
                            b = bnd_base[k]
                            src = big[:, b : b + 896]
                            L = 896
                        else:
                            src = big[:, SIG0 + j0 : SIG0 + j1]
                            L = j1 - j0
                        last = vector.tensor_scalar(
                            scr_v[:, :L],
                            src,
                            rho[:, k : k + 1],
                            0.0,
                            Alu.is_gt,
                            Alu.add,
                            accum_out=a,
                        )
                assert last is not None
                last.then_inc(vdone, 1)

            if use_scalare:

                @block.scalar
                def _(scalar):
                    waited = set()

                    def need(c0, c1):
                        for g in range(grp_of(c0), grp_of(c1) + 1):
                            if g not in waited:
                                waited.add(g)
                                scalar.wait_ge(g_sem[g], 16)

                    scalar.wait_ge(rho_done, 1)
                    scalar.activation(
                        out=rhon[:], in_=rho[:], func=Act.Copy, scale=-1.0
                    )
                    last = None
                    for w in s_work:
                        kind, k, j0, j1, eng = w
                        idx = work.index(w)
                        a = acc[:, idx : idx + 1]
                        need(*col_range_of(w))
                        if kind == "bndF":
                            b = bnd_base[k]
                            src = big[:, b : b + 896]
                            L = 896
                        else:
                            src = big[:, SIG0 + j0 : SIG0 + j1]
                            L = j1 - j0
                        last = scalar.activation(
                            out=scr_s[:, :L],
                            in_=src,
                            func=Act.Sign,
                            bias=rhon[:, k : k + 1],
                            scale=1.0,
                            accum_out=a,
                        )
                    assert last is not None
                    last.then_inc(sdone, 1)

    return nc


def _prepare(risk, time, event, cfg):
    order = np.argsort(time, kind="stable")
    r = np.asarray(risk)[order]
    e = np.asarray(event)[order]

    # tie-safe ranks: equal risks share a rank so strict is_gt stays exact
    rk = np.searchsorted(np.sort(r), r, side="left").astype(np.int32)
    has_ties = bool(np.unique(r).size != r.size)

    enc_bits = (ENC_BASE + rk).astype(np.uint16)
    sig_bits = np.where(e > 0, enc_bits, np.uint16(0))  # [N] uint16

    # rho[p, k] for core c: row i = 1024k + 128c + p
    rho_all = enc_bits.reshape(NSLOTS, NCORES, 128)  # [k, c, p]

    ship = sorted(cfg["ship"])
    tot = _tot_cols(cfg)
    p_idx = np.arange(128)[:, None]
    jj128 = np.arange(128)[None, :]
    jj1024 = np.arange(CHUNK)[None, :]
    one = np.uint16(0x3F80)  # bf16 1.0 bit pattern

    in_maps = []
    for c in range(NCORES):
        pk = np.zeros((128, tot), dtype=np.uint16)
        pk[:, 0:NSLOTS] = rho_all[:, c, :].T
        pk[:, 8:136] = (jj128 < p_idx).astype(np.uint16) * one
        pk[:, 136:SIG0] = (jj1024 < 128 * c + p_idx).astype(np.uint16) * one
        pk[:, SIG0:BND0] = sig_bits[None, :]
        w = 128 * c
        for f, k in enumerate(ship):
            b = BND0 + 1024 * f
            pk[:, b : b + w] = sig_bits[None, k * CHUNK : k * CHUNK + w]
            pk[:, b + 896 : b + 1024] = sig_bits[
                None, k * CHUNK + w : k * CHUNK + w + 128
            ]
        in_maps.append({"packed": pk.view(BF16)})

    den = float(np.sum(e.astype(np.float64) * (N - 1 - np.arange(N))))
    return in_maps, den, has_ties


def _reduce(results, work):
    num = 0.0
    for rmap in results:
        a = rmap["acc"].astype(np.float64)  # [128, nacc]
        for idx, (kind, k, j0, j1, eng) in enumerate(work):
            col = a[:, idx]
            if eng == "s":
                num += float(np.sum(col + (j1 - j0)) / 2.0)
            else:
                num += float(np.sum(col))
    return num


def kernel(risk, time, event, _trace=False, _cfg=None):
    from concourse.bass_utils import run_bass_kernel_spmd

    cfg = dict(DEFAULT_CFG)
    if _cfg:
        cfg.update(_cfg)
    in_maps, den, has_ties = _prepare(risk, time, event, cfg)
    if has_ties:
        cfg["scalare_chunks"] = {}  # Sign trick miscounts exact ties by 0.5
        cfg["scalare_bndf"] = frozenset()
    work = _build_work(cfg)
    use_scalare = any(w[4] == "s" for w in work)
    if cfg.get("raw", True):
        nc = _build_program_raw(work, cfg, use_scalare)
    else:
        nc = _build_program(work, cfg, use_scalare, funnels=True)
    res = run_bass_kernel_spmd(nc, in_maps, list(range(NCORES)), trace=_trace)
    num = _reduce(res.results, work)

    if den == 0.0:
        out = np.float32(np.nan)
    else:
        out = np.float32(num / den)
    if _trace:
        return np.asarray(out, dtype=np.float32), res
    return np.asarray(out, dtype=np.float32)


# revision 53
# speedup vs baseline: 1.1099x; 1.0158x over previous
"""Concordance index kernel for Trainium2 (8 NeuronCores, Bass/Tile).

Math: reference sorts by time (stable), then
  num = sum_i #{ j < i : event_j and risk_j > risk_i }   (i, j in time order)
  den = sum_p e_p * (n-1-p)
  out = num / den

Device computes num (the O(n^2) pairwise part). Host does the O(n log n)
prep: argsort by time, risk ranks, den, and data layout.

Encodings / decomposition:
- risk values -> tie-safe ranks (equal values share a rank), encoded as bf16
  via bit pattern (16384 + rank): strictly monotone, so bf16 `is_gt`
  compares are EXACT, and bf16 enables DVE's 4x perf mode.
- event mask fused into the comparison: sigma_j = event_j ? enc(rank_j) : 0.0
  (0.0 sorts below every encoded rank, so non-events never count).
- row i = 1024*k + 128*c + p  ->  core c, slot k, partition p.  Every core
  runs an IDENTICAL instruction schedule (SPMD + perfect balance).
  The prefix j < i of slot k splits into:
    main(k):  j in [0, 1024k)  unmasked tensor_scalar(is_gt)+accum at 4x
              (DVE) or activation(Sign)+accum (ScalarE, count=(S+N)/2)
    boundary j in [1024k, 1024k + 128c + p), handled one of two ways:
      k in SHIP set: bndF = unmasked 4x count over a shipped per-core
        periodically-zeroed sigma copy (zeros at jj >= 128c, fixed 896-col
        AP) + bndT = 128-wide triangle STT with mask 1[jj < p];
      else: one scalar_tensor_tensor((sig is_gt rho) * stair) at 1x with
        the staircase mask 1[jj < 128c + p].
  Trade-off: shipping bndF data costs +0.25MB DMA per slot but turns a 1x
  1024-col op into a 4x 896-col op + tiny triangle. The SHIP set balances
  the DMA-vs-DVE critical path.
- per-instruction [128,1] fp32 partials are integers; host sums in float64.

Packed per-core input layout (bf16 columns):
  [0:8]          rho_bf (slot k threshold at col k, per partition)
  [8:136]        triangle mask 1[jj < p]
  [136:1160]     staircase mask 1[jj < 128c + p]
  [1160:9352]    sigma[0:8192] (contiguous -> simple main APs)
  [9352 + 1024f] bndF_k(896) | bndT_k(128) for the f-th slot of SHIP

Hardware-constraint notes (hard-won):
- Most instruction formats hold ONE sem wait (DMA copies, control/drain) or
  very few (compute). So: all inputs in ONE tensor moved by <=7 dma_starts
  (+1 output = 8 queues max, fresh queue for the output), compute issued in
  DMA-arrival order (each op adds <=1 new wait), per-engine 1-column
  "funnel" copies collapse DMA-queue sems into program order, SP nops with
  explicit dep edges pre-consume queue/engine sems so the kernel-tail drain
  stays within its wait budget, and the output DMA reads a tile written by
  exactly one instruction.
- GPSIMD cannot execute TensorScalarPtr (ISA check) - no compute there.
"""

import os
import sys

import numpy as np

for _p in ("/opt/trn_rl_repo", "/root/.axon_site/_ro/trn_rl_repo"):
    if os.path.isdir(_p) and _p not in sys.path:
        sys.path.insert(0, _p)

import ml_dtypes  # noqa: E402

N = 8192
NCORES = 8
NSLOTS = 8  # row groups per core; group g = 8*k + c; 128 rows per group
CHUNK = 1024  # j-columns per slot
BF16 = ml_dtypes.bfloat16
ENC_BASE = 16384  # bf16 bit pattern base (value 2.0); +8191 stays finite

SIG0 = 8 + 128 + CHUNK  # header: rho | tri_mask | stair_mask
BND0 = SIG0 + N  # 9352: start of shipped boundary data

# default engine config; tuned via TimelineSim + HW checks
DEFAULT_CFG = {
    # main-slot chunks on ScalarE: k -> number of leading 1024-chunks
    "scalare_chunks": {7: 7, 6: 2},
    "s_merge_from": 2,  # ScalarE chunks >= this index merge into one op
    "ship": (2, 3, 4, 5, 6, 7),  # slots with shipped bndF/bndT data
    "scalare_bndf": frozenset({7}),  # bndF slots on ScalarE (Sign trick)
    "raw": True,  # raw Block program (no Tile scheduling/tail overhead)
    # input DMA split (ramp-friendly: small first group); None -> heuristic
    "grp_ends": (
        SIG0 + 512,
        SIG0 + 1536,
        SIG0 + 2560,
        SIG0 + 3584,
        SIG0 + 5120,
        SIG0 + 6656,
        BND0,
        BND0 + 2048,
        BND0 + 4096,
    ),
}


def _tot_cols(cfg):
    return BND0 + 1024 * len(cfg["ship"])


def _grp_ends(cfg):
    """<=7 input DMA groups: sigma split for pipelining, then bnd data."""
    tot = _tot_cols(cfg)
    if cfg.get("grp_ends"):
        return tuple(min(e, tot) for e in cfg["grp_ends"] if e <= tot) + (
            (tot,) if cfg["grp_ends"][-1] < tot else ()
        )
    ends = [SIG0 + 1024, SIG0 + 3072, SIG0 + 5120, BND0]
    nb = len(cfg["ship"])
    if nb == 0:
        return tuple(ends)
    if nb >= 4:
        ends += [BND0 + 1024 * (nb // 2), tot]
    else:
        ends += [tot]
    return tuple(ends)


def _build_work(cfg):
    """Instruction list: (kind, k, j0, j1, eng) in pipeline issue order.

    kind: 'main' | 'bndF' | 'bndT' | 'bndS' (staircase STT).
    eng: 'v' DVE, 's' ScalarE.
    """
    ship = set(cfg["ship"])
    merge_from = cfg.get("s_merge_from", NSLOTS)  # chunks >= this merge
    entries = []
    for k in range(1, NSLOTS):
        ns = min(cfg["scalare_chunks"].get(k, 0), k)
        for u in range(min(ns, merge_from)):  # per-chunk pieces (pipeline)
            entries.append(((u, 2), ("main", k, u * CHUNK, (u + 1) * CHUNK, "s")))
        if ns > merge_from:  # tail chunks merged into one ACT op
            entries.append(
                ((merge_from, 2), ("main", k, merge_from * CHUNK, ns * CHUNK, "s"))
            )
        if ns < k:
            entries.append(((k - 1, 3), ("main", k, ns * CHUNK, k * CHUNK, "v")))
    for k in range(NSLOTS):
        if k in ship:
            eF = "s" if k in cfg["scalare_bndf"] else "v"
            entries.append(((7, 4, k), ("bndF", k, 0, 896, eF)))
            entries.append(((7, 5, k), ("bndT", k, 0, 128, "v")))
        else:
            entries.append(((k, 1), ("bndS", k, 0, CHUNK, "v")))
    entries.sort(key=lambda e: e[0])
    return [e for _, e in entries]


def _build_program(work, cfg, use_scalare, skip_compute=False, funnels=True):
    import bass_rust
    import concourse.bass as bass
    import concourse.mybir as mybir
    from concourse.tile import TileContext

    dt = mybir.dt
    Alu = mybir.AluOpType
    Act = mybir.ActivationFunctionType

    ship = sorted(cfg["ship"])
    bnd_base = {k: BND0 + 1024 * f for f, k in enumerate(ship)}
    grp_ends = _grp_ends(cfg)
    tot = _tot_cols(cfg)
    nacc = len(work)
    ngrp = len(grp_ends)
    nc = bass.Bass()
    packed_d = nc.declare_dram_parameter("packed", [128, tot], dt.bfloat16, False)
    acc_d = nc.declare_dram_parameter("acc", [128, nacc], dt.float32, True)

    with TileContext(nc) as tc:
        with tc.tile_pool(name="p", bufs=1) as pool:
            big = pool.tile([128, tot], dt.bfloat16)
            rho = pool.tile([128, NSLOTS], dt.float32)
            rhon = pool.tile([128, NSLOTS], dt.float32)
            acc = pool.tile([128, nacc], dt.float32)
            acc2 = pool.tile([128, nacc], dt.float32)
            scr_v = pool.tile([128, (NSLOTS - 1) * CHUNK], dt.bfloat16)
            scr_s = pool.tile([128, (NSLOTS - 1) * CHUNK], dt.bfloat16)
            warm_v = pool.tile([128, ngrp], dt.bfloat16)
            warm_s = pool.tile([128, ngrp], dt.bfloat16)

            tri_mask = big[:, 8 : 8 + 128]
            stair = big[:, 136 : 136 + CHUNK]

            g0 = 0
            dmas = []
            for ge in grp_ends:
                dmas.append(
                    nc.sync.dma_start(out=big[:, g0:ge], in_=packed_d[:, g0:ge])
                )
                g0 = ge

            # per-engine funnels: collapse each DMA group's queue sem into
            # the engine's program order via a 1-column copy
            funneled = {e: [not funnels] * ngrp for e in ("v", "s")}
            warms = {"v": warm_v, "s": warm_s}

            def _funnel(col_abs, eng):
                flags = funneled[eng]
                for g in range(ngrp):
                    gstart = 0 if g == 0 else grp_ends[g - 1]
                    if flags[g] or gstart > col_abs:
                        continue
                    flags[g] = True
                    c = grp_ends[g] - 1
                    if eng == "s":
                        nc.scalar.copy(warms[eng][:, g : g + 1], big[:, c : c + 1])
                    else:
                        nc.vector.tensor_copy(
                            warms[eng][:, g : g + 1], big[:, c : c + 1]
                        )

            # rho (fp32, for is_gt scalar / Sign bias) via converting copy
            _funnel(SIG0, "v")
            nc.vector.tensor_copy(rho[:], big[:, 0:NSLOTS])
            if use_scalare:
                _funnel(SIG0, "s")
                nc.scalar.activation(
                    out=rhon[:], in_=rho[:], func=Act.Copy, scale=-1.0
                )

            last_inst_by_eng = {}
            if skip_compute:
                nc.vector.memset(acc[:], 0.0)
            for idx, (kind, k, j0, j1, eng) in enumerate(work):
                if skip_compute:
                    break
                a = acc[:, idx : idx + 1]
                if kind == "bndT":
                    b = bnd_base[k]
                    _funnel(b + 1024 - 1, "v")
                    inst = nc.vector.scalar_tensor_tensor(
                        out=scr_v[:, :128],
                        in0=big[:, b + 896 : b + 1024],
                        scalar=rho[:, k : k + 1],
                        in1=tri_mask,
                        op0=Alu.is_gt,
                        op1=Alu.mult,
                        accum_out=a,
                    )
                elif kind == "bndS":
                    _funnel(SIG0 + (k + 1) * CHUNK - 1, "v")
                    inst = nc.vector.scalar_tensor_tensor(
                        out=scr_v[:, :CHUNK],
                        in0=big[:, SIG0 + k * CHUNK : SIG0 + (k + 1) * CHUNK],
                        scalar=rho[:, k : k + 1],
                        in1=stair,
                        op0=Alu.is_gt,
                        op1=Alu.mult,
                        accum_out=a,
                    )
                else:
                    if kind == "bndF":
                        b = bnd_base[k]
                        src = big[:, b : b + 896]
                        last_col = b + 896 - 1
                    else:
                        src = big[:, SIG0 + j0 : SIG0 + j1]
                        last_col = SIG0 + j1 - 1
                    L = j1 - j0
                    _funnel(last_col, eng)
                    if eng == "v":
                        inst = nc.vector.tensor_scalar(
                            scr_v[:, :L],
                            src,
                            rho[:, k : k + 1],
                            0.0,
                            Alu.is_gt,
                            Alu.add,
                            accum_out=a,
                        )
                    else:  # ScalarE Sign trick; count = (S + L)/2 host-side
                        inst = nc.scalar.activation(
                            out=scr_s[:, :L],
                            in_=src,
                            func=Act.Sign,
                            bias=rhon[:, k : k + 1],
                            scale=1.0,
                            accum_out=a,
                        )
                last_inst_by_eng[eng] = inst

            # single-writer funnel so the output DMA needs exactly one wait;
            # pre-consume the ScalarE completion sem first (1 wait per copy)
            for e in last_inst_by_eng:
                if e != "v":
                    nc.vector.tensor_copy(acc2[:, :1], acc[:, :1])
            nc.vector.tensor_copy(acc2[:], acc[:])
            dmas.append(nc.sync.dma_start(out=acc_d[:], in_=acc2[:]))

            # kernel-tail drain holds very few waits: pre-consume queue and
            # engine sems on the SP proc via nops with explicit dep edges
            for e, inst in last_inst_by_eng.items():
                if e != "v":
                    nop = nc.sync.nop(nofuse=True)
                    bass_rust.add_dep_helper(
                        nop.ins, inst.ins, reason="spread drain engine waits"
                    )
            for d in dmas:
                nop = nc.sync.nop(nofuse=True)
                bass_rust.add_dep_helper(
                    nop.ins, d.ins, reason="spread drain queue waits"
                )
    return nc


def _build_program_raw(work, cfg, use_scalare):
    """Raw Block-mode program: explicit per-engine streams + semaphores.

    Skips TileContext's scheduling and its expensive kernel-tail drain +
    barrier. Each `wait_ge` is its own instruction, so the tiny per-format
    sem-wait budgets stop mattering. One semaphore per input DMA group
    (queue completions are out-of-order), plus rho-staging and per-engine
    completion sems gating the output DMA.
    """
    import concourse.bass as bass
    import concourse.mybir as mybir

    dt = mybir.dt
    Alu = mybir.AluOpType
    Act = mybir.ActivationFunctionType

    ship = sorted(cfg["ship"])
    bnd_base = {k: BND0 + 1024 * f for f, k in enumerate(ship)}
    grp_ends = _grp_ends(cfg)
    tot = _tot_cols(cfg)
    nacc = len(work)
    ngrp = len(grp_ends)

    nc = bass.Bass()
    packed_d = nc.declare_dram_parameter("packed", [128, tot], dt.bfloat16, False)
    acc_d = nc.declare_dram_parameter("acc", [128, nacc], dt.float32, True)

    with (
        nc.sbuf_tensor("big", [128, tot], dt.bfloat16) as big,
        nc.sbuf_tensor("rho", [128, NSLOTS], dt.float32) as rho,
        nc.sbuf_tensor("rhon", [128, NSLOTS], dt.float32) as rhon,
        nc.sbuf_tensor("acc_sb", [128, nacc], dt.float32) as acc,
        nc.sbuf_tensor("scr_v", [128, (NSLOTS - 1) * CHUNK], dt.bfloat16) as scr_v,
        nc.sbuf_tensor("scr_s", [128, (NSLOTS - 1) * CHUNK], dt.bfloat16) as scr_s,
    ):
        sems = [nc.semaphore(f"g{g}") for g in range(ngrp)]
        g_sem = [s.__enter__() for s in sems]
        rho_done = nc.semaphore("rho_done").__enter__()
        vdone = nc.semaphore("vdone").__enter__()
        sdone = nc.semaphore("sdone").__enter__()
        odone = nc.semaphore("odone").__enter__()

        def grp_of(col):
            for g, ge in enumerate(grp_ends):
                if col < ge:
                    return g
            return ngrp - 1

        v_work = [w for w in work if w[4] == "v"]
        s_work = [w for w in work if w[4] == "s"]

        def col_range_of(w):
            """(first, last) input columns an op reads (besides rho)."""
            kind, k, j0, j1, eng = w
            if kind == "bndT":
                return (8, bnd_base[k] + 1024 - 1)  # tri mask + bndT cols
            if kind == "bndF":
                return (bnd_base[k], bnd_base[k] + 896 - 1)
            if kind == "bndS":
                return (136, SIG0 + (k + 1) * CHUNK - 1)  # stair + chunk
            return (SIG0 + j0, SIG0 + j1 - 1)

        with nc.Block() as block:

            @block.sync
            def _(sync):
                spans = []
                g0 = 0
                for ge in grp_ends:
                    spans.append((g0, ge))
                    g0 = ge
                order = cfg.get("dma_order") or range(ngrp)
                for g in order:
                    a0, a1 = spans[g]
                    sync.dma_start(
                        out=big[:, a0:a1], in_=packed_d[:, a0:a1]
                    ).then_inc(g_sem[g], 16)
                sync.wait_ge(vdone, 1)
                if use_scalare:
                    sync.wait_ge(sdone, 1)
                sync.dma_start(out=acc_d[:], in_=acc[:]).then_inc(odone, 16)
                sync.wait_ge(odone, 16)

            @block.vector
            def _(vector):
                waited = set()

                def need(c0, c1):
                    for g in range(grp_of(c0), grp_of(c1) + 1):
                        if g not in waited:
                            waited.add(g)
                            vector.wait_ge(g_sem[g], 16)

                need(0, 0)
                vector.tensor_copy(rho[:], big[:, 0:NSLOTS]).then_inc(rho_done, 1)
                last = None
                for w in v_work:
                    kind, k, j0, j1, eng = w
                    idx = work.index(w)
                    a = acc[:, idx : idx + 1]
                    need(*col_range_of(w))
                    if kind == "bndT":
                        b = bnd_base[k]
                        last = vector.scalar_tensor_tensor(
                            out=scr_v[:, :128],
                            in0=big[:, b + 896 : b + 1024],
                            scalar=rho[:, k : k + 1],
                            in1=big[:, 8 : 8 + 128],
                            op0=Alu.is_gt,
                            op1=Alu.mult,
                            accum_out=a,
                        )
                    elif kind == "bndS":
                        last = vector.scalar_tensor_tensor(
                            out=scr_v[:, :CHUNK],
                            in0=big[:, SIG0 + k * CHUNK : SIG0 + (k + 1) * CHUNK],
                            scalar=rho[:, k : k + 1],
                            in1=big[:, 136 : 136 + CHUNK],
                            op0=Alu.is_gt,
                            op1=Alu.mult,
                            accum_out=a,
                        )
                    else:
                        if kind == "bndF":
                            b = bnd_base[k]
                            src = big[:, b : b + 896]
                            L = 896
                        else:
                            src = big[:, SIG0 + j0 : SIG0 + j1]
                            L = j1 - j0
                        last = vector.tensor_scalar(
                            scr_v[:, :L],
                            src,
                            rho[:, k : k + 1],
                            0.0,
                            Alu.is_gt,
                            Alu.add,
                            accum_out=a,
                        )
                assert last is not None
                last.then_inc(vdone, 1)

            if use_scalare:

                @block.scalar
                def _(scalar):
                    waited = set()

                    def need(c0, c1):
                        for g in range(grp_of(c0), grp_of(c1) + 1):
                            if g not in waited:
                                waited.add(g)
                                scalar.wait_ge(g_sem[g], 16)

                    scalar.wait_ge(rho_done, 1)
                    scalar.activation(
                        out=rhon[:], in_=rho[:], func=Act.Copy, scale=-1.0
                    )
                    last = None
                    for w in s_work:
                        kind, k, j0, j1, eng = w
                        idx = work.index(w)
                        a = acc[:, idx : idx + 1]
                        need(*col_range_of(w))
                        if kind == "bndF":
                            b = bnd_base[k]
                            src = big[:, b : b + 896]
                            L = 896
                        else:
                            src = big[:, SIG0 + j0 : SIG0 + j1]
                            L = j1 - j0
                        last = scalar.activation(
                            out=scr_s[:, :L],
                            in_=src,
                            func=Act.Sign,
                            bias=rhon[:, k : k + 1],
                            scale=1.0,
                            accum_out=a,
                        )
                    assert last is not None
                    last.then_inc(sdone, 1)

    return nc


def _prepare(risk, time, event, cfg):
    order = np.argsort(time, kind="stable")
    r = np.asarray(risk)[order]
    e = np.asarray(event)[order]

    # tie-safe ranks: equal risks share a rank so strict is_gt stays exact
    rk = np.searchsorted(np.sort(r), r, side="left").astype(np.int32)
    has_ties = bool(np.unique(r).size != r.size)

    enc_bits = (ENC_BASE + rk).astype(np.uint16)
    sig_bits = np.where(e > 0, enc_bits, np.uint16(0))  # [N] uint16

    # rho[p, k] for core c: row i = 1024k + 128c + p
    rho_all = enc_bits.reshape(NSLOTS, NCORES, 128)  # [k, c, p]

    ship = sorted(cfg["ship"])
    tot = _tot_cols(cfg)
    p_idx = np.arange(128)[:, None]
    jj128 = np.arange(128)[None, :]
    jj1024 = np.arange(CHUNK)[None, :]
    one = np.uint16(0x3F80)  # bf16 1.0 bit pattern

    in_maps = []
    for c in range(NCORES):
        pk = np.zeros((128, tot), dtype=np.uint16)
        pk[:, 0:NSLOTS] = rho_all[:, c, :].T
        pk[:, 8:136] = (jj128 < p_idx).astype(np.uint16) * one
        pk[:, 136:SIG0] = (jj1024 < 128 * c + p_idx).astype(np.uint16) * one
        pk[:, SIG0:BND0] = sig_bits[None, :]
        w = 128 * c
        for f, k in enumerate(ship):
            b = BND0 + 1024 * f
            pk[:, b : b + w] = sig_bits[None, k * CHUNK : k * CHUNK + w]
            pk[:, b + 896 : b + 1024] = sig_bits[
                None, k * CHUNK + w : k * CHUNK + w + 128
            ]
        in_maps.append({"packed": pk.view(BF16)})

    den = float(np.sum(e.astype(np.float64) * (N - 1 - np.arange(N))))
    return in_maps, den, has_ties


def _reduce(results, work):
    num = 0.0
    for rmap in results:
        a = rmap["acc"].astype(np.float64)  # [128, nacc]
        for idx, (kind, k, j0, j1, eng) in enumerate(work):
            col = a[:, idx]
            if eng == "s":
                num += float(np.sum(col + (j1 - j0)) / 2.0)
            else:
                num += float(np.sum(col))
    return num


def kernel(risk, time, event, _trace=False, _cfg=None):
    from concourse.bass_utils import run_bass_kernel_spmd

    cfg = dict(DEFAULT_CFG)
    if _cfg:
        cfg.update(_cfg)
    in_maps, den, has_ties = _prepare(risk, time, event, cfg)
    if has_ties:
        cfg["scalare_chunks"] = {}  # Sign trick miscounts exact ties by 0.5
        cfg["scalare_bndf"] = frozenset()
    work = _build_work(cfg)
    use_scalare = any(w[4] == "s" for w in work)
    if cfg.get("raw", True):
        nc = _build_program_raw(work, cfg, use_scalare)
    else:
        nc = _build_program(work, cfg, use_scalare, funnels=True)
    res = run_bass_kernel_spmd(nc, in_maps, list(range(NCORES)), trace=_trace)
    num = _reduce(res.results, work)

    if den == 0.0:
        out = np.float32(np.nan)
    else:
        out = np.float32(num / den)
    if _trace:
        return np.asarray(out, dtype=np.float32), res
    return np.asarray(out, dtype=np.float32)
